# revision 1
# baseline (speedup 1.0000x reference)
"""Epipolar (KNN-sparse) attention on 8 Trainium2 NeuronCores — v2.

Sharding v2 (query-parallel): 8 cores = 2 batches x 4 query-quarters.
Each core handles 1024 queries x ALL 8 heads, so gathered kv rows are
full-width (2KB) -> 4x fewer gather descriptors than the head-sharded v1
(SWDGE descriptor generation on the Pool engine was the v1 bottleneck:
~7.9ns/row on one Q7 pair).

Per core:
  phase 1: project FULL kv table (4096 targets x 512ch k + 512ch v, bf16)
           into kv_dram rows [k(512)||v(512)] = 1024 bf16 = 2KB.
  phase 2: per 128-query tile (8 tiles): q-proj (PE), dma_gather of
           128*32 full rows (2 sub-gathers of 2048 idxs so the drain of
           sub-gather 0 overlaps gen of sub-gather 1, alternating SWDGE
           queues), qk product + d-tree + f32 reduce + pair-weight add
           (DVE, contiguous scratch for the 2x_1P bf16 uop), exp written
           twice as adjacent pairs (ACT; keeps the v-weighting in 2x_1P),
           denominators (DVE reduce), v weighting + j-tree (DVE),
           per-head 1/den scale (ACT), transpose + Wo projection (PE),
           store full out rows. k/v biases are never applied on-device:
           q.bk is j-constant so it cancels in softmax; bv folds into
           bo' = bo + bv @ Wo host-side.

Host: sorts each query's neighbor list (locality for the gather), permutes
pair weights identically, precasts all dense inputs to bf16, folds SCALE
into Wq/bq. Output assembly is pure concatenation (no reduction).
"""

import sys

import numpy as np

sys.path.insert(0, "/opt/trn_rl_repo")

from contextlib import ExitStack

import ml_dtypes

import concourse.bass as bass
import concourse.tile as tile
from concourse import bacc, masks, mybir
from concourse.bass_utils import run_bass_kernel_spmd

F32 = mybir.dt.float32
BF16 = mybir.dt.bfloat16
I16 = mybir.dt.int16
AF = mybir.ActivationFunctionType
OP = mybir.AluOpType

B, HW, NTGT, C = 2, 4096, 4096, 512
H, KNN = 8, 32
DH = C // H
SCALE = DH ** -0.5
P = 128
QL = HW // 4            # queries per core (1024)
NT = QL // P            # query tiles per core (8)
NTT = NTGT // P         # target tiles (32)
CK = C // P             # contraction chunks (4)
ROW = 2 * C             # kv row elems (1024 bf16 = 2KB)
NSUB = 2                # sub-gathers per query tile
JSUB = KNN // NSUB      # neighbors per sub-gather


def build_program():
    nc = bacc.Bacc("TRN2", target_bir_lowering=False, debug=False,
                   num_devices=8, num_swdge_queues=2)

    srcT = nc.dram_tensor("srcT", (C, QL), BF16, kind="ExternalInput").ap()
    tgtT = nc.dram_tensor("tgtT", (C, NTGT), BF16, kind="ExternalInput").ap()
    wq = nc.dram_tensor("wq", (C, C), BF16, kind="ExternalInput").ap()
    wk = nc.dram_tensor("wk", (C, C), BF16, kind="ExternalInput").ap()
    wv = nc.dram_tensor("wv", (C, C), BF16, kind="ExternalInput").ap()
    wo = nc.dram_tensor("wo", (C, C), BF16, kind="ExternalInput").ap()
    bq = nc.dram_tensor("bq", (1, C), BF16, kind="ExternalInput").ap()
    bo = nc.dram_tensor("bo", (1, C), BF16, kind="ExternalInput").ap()
    idxw = nc.dram_tensor("idxw", (NT, P, KNN * P // 16), I16,
                          kind="ExternalInput").ap()
    wts = nc.dram_tensor("wts", (QL, KNN), F32, kind="ExternalInput").ap()
    out = nc.dram_tensor("out", (QL, C), F32, kind="ExternalOutput").ap()

    with tile.TileContext(nc) as tc, ExitStack() as ctx:
        tp = lambda name, bufs, **kw: ctx.enter_context(
            tc.tile_pool(name=name, bufs=bufs, **kw))

        cpool = tp("consts", 1)
        dram = tp("dram", 1, space="DRAM")
        kv_dram = dram.tile([NTGT, ROW], BF16)

        ident = cpool.tile([P, P], BF16, tag="ident")
        masks.make_identity(nc, ident[:])
        ones = cpool.tile([1, P], BF16, tag="ones")
        nc.gpsimd.memset(ones[:], 1.0)

        # resident weights/biases (host-precast bf16)
        wq_sb = cpool.tile([P, CK * C], BF16, tag="wq")
        wo_sb = cpool.tile([P, CK * C], BF16, tag="wo")
        bq_sb = cpool.tile([1, C], BF16, tag="bq")
        bo_sb = cpool.tile([1, C], BF16, tag="bo")
        srcT_sb = cpool.tile([P, CK * QL], BF16, tag="srcT")
        for c in range(CK):
            nc.sync.dma_start(wq_sb[:, c * C:(c + 1) * C],
                              wq[c * P:(c + 1) * P, :])
            nc.sync.dma_start(wo_sb[:, c * C:(c + 1) * C],
                              wo[c * P:(c + 1) * P, :])
            nc.sync.dma_start(srcT_sb[:, c * QL:(c + 1) * QL],
                              srcT[c * P:(c + 1) * P, :])
        nc.sync.dma_start(bq_sb[:1, :], bq[:, :])
        nc.sync.dma_start(bo_sb[:1, :], bo[:, :])

        # ---- phase 1: k/v projection of the FULL target table ----
        # wkv holds Wk||Wv side by side so each target tile needs only 4
        # accumulating N=1024 matmuls (one per contraction chunk).
        with tc.tile_pool(name="p1w", bufs=1) as p1w, \
             tc.tile_pool(name="p1psum", bufs=2, space="PSUM") as p1ps, \
             tc.tile_pool(name="p1out", bufs=3) as p1out:
            # No k/v biases on-device: q·bk is constant over a query's
            # neighbors so it cancels in the softmax, and bv passes through
            # the softmax (weights sum to 1) and is folded host-side into
            # bo' = bo + bv @ Wo. Saves 64 PE bias-matmuls in phase 1.
            wkv_sb = p1w.tile([P, CK * ROW], BF16, tag="wkv")
            tgtT_sb = p1w.tile([P, CK * NTGT], BF16, tag="tgtT")
            for c in range(CK):
                nc.sync.dma_start(wkv_sb[:, c * ROW:c * ROW + C],
                                  wk[c * P:(c + 1) * P, :])
                nc.sync.dma_start(wkv_sb[:, c * ROW + C:(c + 1) * ROW],
                                  wv[c * P:(c + 1) * P, :])
                nc.sync.dma_start(tgtT_sb[:, c * NTGT:(c + 1) * NTGT],
                                  tgtT[c * P:(c + 1) * P, :])

            for t in range(NTT):
                pskv = p1ps.tile([P, ROW], F32, tag="pskv")
                for half in range(2):
                    hs = half * C
                    for c in range(CK):
                        lhsT = tgtT_sb[:, c * NTGT + t * P:
                                       c * NTGT + (t + 1) * P]
                        nc.tensor.matmul(pskv[:, hs:hs + C], lhsT,
                                         wkv_sb[:, c * ROW + hs:
                                                c * ROW + hs + C],
                                         start=(c == 0), stop=(c == CK - 1))
                kv_sb = p1out.tile([P, ROW], BF16, tag="kv")
                nc.scalar.copy(kv_sb[:], pskv[:])
                nc.sync.dma_start(kv_dram[t * P:(t + 1) * P, :], kv_sb[:])

        # ---- phase 2: per-query-tile sparse attention ----
        qps = tp("qpsum", 2, space="PSUM")
        tps = tp("tpsum", 2, space="PSUM")
        ops_pool = tp("opsum", 2, space="PSUM")
        small = tp("small", 2)
        small3 = tp("small3", 3)
        gat = tp("gather", 2)
        scr = tp("scratch", 1)
        scr2 = tp("scratch2", 2)
        outp = tp("outstage", 2)

        st = {}

        def emit_gathers(t, kvg, idx_sb, prepare=False):
            """Two sub-gathers on alternating SWDGE queues (separate rings
            and Q7 pairs decouple ring back-pressure between the subs)."""
            for sub in range(NSUB):
                kw = {}
                if prepare:
                    kw = dict(prepare_only=True,
                              sem=nc.alloc_semaphore(f"pregather{sub}"))
                nc.gpsimd.dma_gather(
                    kvg[:, sub * JSUB * ROW:(sub + 1) * JSUB * ROW]
                        .rearrange("p (j d) -> p j d", j=JSUB),
                    kv_dram[:, :],
                    idx_sb[:, sub * JSUB * P // 16:(sub + 1) * JSUB * P // 16],
                    num_idxs=JSUB * P,
                    num_idxs_reg=JSUB * P,
                    elem_size=ROW,
                    single_packet=False,
                    queue_num=sub,
                    **kw,
                )

        def stage_a(t, emit_gather=True):
            """q-proj + idx/wts loads + gather gen (Pool)."""
            s = st.get(t, {})
            psq = qps.tile([P, C], F32, tag="psq")
            for c in range(CK):
                nc.tensor.matmul(
                    psq[:], srcT_sb[:, c * QL + t * P: c * QL + (t + 1) * P],
                    wq_sb[:, c * C:(c + 1) * C],
                    start=(c == 0), stop=False)
            nc.tensor.matmul(psq[:], ones[:1, :], bq_sb[:1, :],
                             start=False, stop=True)
            q_sb = small.tile([P, C], BF16, tag="q")
            nc.scalar.copy(q_sb[:], psq[:])
            s["q"] = q_sb

            w_sb = small.tile([P, KNN], F32, tag="w")
            nc.sync.dma_start(w_sb[:], wts[t * P:(t + 1) * P, :])
            s["w"] = w_sb

            if emit_gather:
                idx_sb = small.tile([P, KNN * P // 16], I16, tag="idx")
                nc.sync.dma_start(idx_sb[:], idxw[t, :, :])
                kvg = gat.tile([P, KNN * ROW], BF16, tag="kvg")
                emit_gathers(t, kvg, idx_sb)
                s["kvg"] = kvg
            return s

        JH = KNN // 2   # neighbors per compute half

        def stage_b(s):
            """qk product + d-tree + logits + pair weights + exp + 1/den.

            Processed in two j-halves with contiguous scratch outputs (DVE
            runs ~3x faster with contiguous destinations than in-place
            strided writes into the gathered buffer)."""
            kvg3 = s["kvg"][:].rearrange("p (j d) -> p j d", j=KNN)
            logits = small.tile([P, KNN * H], F32, tag="logits")
            qb = s["q"][:].unsqueeze(1).broadcast_to([P, JH, C])
            for jh in range(2):
                kh = kvg3[:, jh * JH:(jh + 1) * JH, 0:C]
                prod = scr.tile([P, JH * C], BF16, tag="prod16")
                prodv = prod[:].rearrange("p (j h e) -> p j h e", j=JH, h=H)
                nc.vector.tensor_tensor(
                    prod[:].rearrange("p (j d) -> p j d", j=JH),
                    kh, qb, op=OP.mult)
                t16 = scr.tile([P, JH * H * (DH // 2)], BF16, tag="t16h")
                t16v = t16[:].rearrange("p (j h e) -> p j h e", j=JH, h=H)
                nc.vector.tensor_tensor(t16v, prodv[:, :, :, 0:DH // 2],
                                        prodv[:, :, :, DH // 2:DH], op=OP.add)
                t8 = scr.tile([P, JH * H * (DH // 4)], BF16, tag="t8h")
                t8v = t8[:].rearrange("p (j h e) -> p j h e", j=JH, h=H)
                nc.vector.tensor_tensor(t8v, t16v[:, :, :, 0:DH // 4],
                                        t16v[:, :, :, DH // 4:DH // 2],
                                        op=OP.add)
                nc.vector.tensor_reduce(
                    logits[:, jh * JH * H:(jh + 1) * JH * H]
                        .rearrange("p (j h) -> p j h", j=JH),
                    t8v, axis=mybir.AxisListType.X, op=OP.add)

            logw = small.tile([P, KNN * H], F32, tag="logw")
            nc.vector.tensor_tensor(
                logw[:].rearrange("p (j h) -> p j h", j=KNN),
                logits[:].rearrange("p (j h) -> p j h", j=KNN),
                s["w"][:].unsqueeze(2).broadcast_to([P, KNN, H]),
                op=OP.add)

            # exp duplicated x2 (adjacent pairs) so the v-weighting's in2
            # has an innermost step-1 pair dim -> DVE 2x_1P packing (a
            # step-0 innermost broadcast falls back to the 1x uop: 8.7us
            # vs 4.4us per half). Two strided ACT writes build the pairs.
            exd = small.tile([P, KNN * H * 2], BF16, tag="exd")
            exd3 = exd[:].rearrange("p (jh t) -> p jh t", t=2)
            nc.scalar.activation(exd3[:, :, 0], logw[:], AF.Exp)
            nc.scalar.activation(exd3[:, :, 1], logw[:], AF.Exp)
            den = small.tile([P, H], F32, tag="den")
            nc.vector.tensor_reduce(
                den[:], exd3[:, :, 0].rearrange("p (j h) -> p h j", j=KNN),
                axis=mybir.AxisListType.X, op=OP.add)
            rec = small3.tile([P, H], F32, tag="rec")
            nc.vector.reciprocal(rec[:], den[:])
            s["exd"], s["rec"] = exd, rec

        def stage_c(s):
            """v weighting + j-tree per half, then combine halves."""
            kvg3 = s["kvg"][:].rearrange("p (j d) -> p j d", j=KNN)
            exd4 = s["exd"][:].rearrange("p (j h t) -> p j h t", j=KNN, h=H)
            vt1 = []
            for jh in range(2):
                vh = kvg3[:, jh * JH:(jh + 1) * JH, C:ROW] \
                    .rearrange("p j (h e t) -> p j h e t", h=H, t=2)
                exb = (exd4[:, jh * JH:(jh + 1) * JH, :, :]
                       .unsqueeze(3).broadcast_to([P, JH, H, DH // 2, 2]))
                vprod = scr.tile([P, JH * C], BF16, tag="prod16")
                nc.vector.tensor_tensor(
                    vprod[:].rearrange("p (j h e t) -> p j h e t",
                                       j=JH, h=H, t=2),
                    vh, exb, op=OP.mult)
                vt_in, jw = vprod, JH
                for lvl, tag in ((0, "t16h"), (1, "t8h"), (2, "vt2"),
                                 (3, f"vt1_{jh}")):
                    jw //= 2
                    pool = scr2 if jw == 1 else scr
                    vt = pool.tile([P, jw * C], BF16, tag=tag)
                    a = vt_in[:].rearrange("p (j d) -> p j d", d=C)
                    nc.vector.tensor_tensor(
                        vt[:].rearrange("p (j d) -> p j d", d=C),
                        a[:, 0:jw, :], a[:, jw:2 * jw, :], op=OP.add)
                    vt_in = vt
                vt1.append(vt_in)
            vtf = scr2.tile([P, C], BF16, tag="vtf")
            nc.vector.tensor_tensor(vtf[:], vt1[0][:], vt1[1][:], op=OP.add)
            s["vtf"] = vtf

        def stage_d(s, t):
            """per-head 1/den scale + out projection + store."""
            vt_in = s["vtf"]
            ao = small.tile([P, C], BF16, tag="ao")
            for h in range(H):
                nc.scalar.activation(
                    ao[:, h * DH:(h + 1) * DH], vt_in[:, h * DH:(h + 1) * DH],
                    AF.Copy, scale=s["rec"][:, h:h + 1])
            aoT_ps = tps.tile([P, C], BF16, tag="aoT")
            for c in range(CK):
                nc.tensor.transpose(aoT_ps[:, c * P:(c + 1) * P],
                                    ao[:, c * P:(c + 1) * P], ident[:])
            aoT = small.tile([P, C], BF16, tag="aoTsb")
            nc.scalar.copy(aoT[:], aoT_ps[:])
            ops = ops_pool.tile([P, C], F32, tag="ops")
            for c in range(CK):
                nc.tensor.matmul(ops[:], aoT[:, c * P:(c + 1) * P],
                                 wo_sb[:, c * C:(c + 1) * C],
                                 start=(c == 0), stop=False)
            nc.tensor.matmul(ops[:], ones[:1, :], bo_sb[:1, :],
                             start=False, stop=True)
            o_sb = outp.tile([P, C], F32, tag="osb")
            nc.scalar.copy(o_sb[:], ops[:])
            nc.sync.dma_start(out[t * P:(t + 1) * P, :], o_sb[:])

        for i in range(NT + 2):
            if i < NT:
                st[i] = stage_a(i)
            if 1 <= i <= NT:
                stage_b(st[i - 1])
                stage_c(st[i - 1])
            if i >= 2:
                stage_d(st[i - 2], i - 2)
                del st[i - 2]

    nc.compile()
    return nc


def _wrap_indices(idx_t):
    """(128, KNN) sorted idx -> [128, 256] int16 wrap (j-major flat,
    16-wrapped, replicated across the 8 gpsimd cores)."""
    flat = idx_t.T.reshape(-1)                      # i = j*128 + q
    wr = flat.reshape(-1, 16).T.astype(np.int16)    # [16, KNN*P/16]
    return np.tile(wr, (8, 1))


_NC_CACHE = {}


def _get_program():
    if "nc" not in _NC_CACHE:
        _NC_CACHE["nc"] = build_program()
    return _NC_CACHE["nc"]


def make_in_maps(src, tgt, indices, weights, Wq, bq, Wk, bk, Wv, bv, Wo, bo):
    f32, bf16 = np.float32, ml_dtypes.bfloat16
    src = np.asarray(src, f32)
    tgt = np.asarray(tgt, f32)
    weights = np.asarray(weights, f32)
    wqs = (np.asarray(Wq, f32) * np.float32(SCALE)).astype(bf16)
    bqs = (np.asarray(bq, f32) * np.float32(SCALE)).astype(bf16)
    wk_b = np.asarray(Wk, f32).astype(bf16)
    wv_b = np.asarray(Wv, f32).astype(bf16)
    wo_b = np.asarray(Wo, f32).astype(bf16)
    # v-bias passes through the softmax (weights sum to 1): fold into bo.
    # The k-bias shifts all of a query's logits equally -> cancels in
    # softmax and is dropped entirely.
    bo_b = (np.asarray(bo, f32)
            + np.asarray(bv, f32) @ np.asarray(Wo, f32)).astype(bf16)

    idx_all = np.asarray(indices)
    in_maps = []
    for core in range(8):
        b, s = divmod(core, 4)
        q0 = s * QL
        idx_b = idx_all[b, q0:q0 + QL]              # (QL, KNN)
        w_b = weights[b, q0:q0 + QL]
        order = np.argsort(idx_b, axis=1, kind="stable")
        idx_s = np.take_along_axis(idx_b, order, axis=1)
        w_s = np.take_along_axis(w_b, order, axis=1)
        idxw_c = np.empty((NT, P, KNN * P // 16), np.int16)
        for t in range(NT):
            idxw_c[t] = _wrap_indices(idx_s[t * P:(t + 1) * P])
        m = {
            "srcT": np.ascontiguousarray(src[b, q0:q0 + QL].T).astype(bf16),
            "tgtT": np.ascontiguousarray(tgt[b].T).astype(bf16),
            "wq": wqs, "wk": wk_b, "wv": wv_b, "wo": wo_b,
            "bq": bqs.reshape(1, C), "bo": bo_b.reshape(1, C),
            "idxw": idxw_c,
            "wts": np.ascontiguousarray(w_s),
        }
        in_maps.append(m)
    return in_maps


def kernel(src, tgt, indices, weights, Wq, bq, Wk, bk, Wv, bv, Wo, bo):
    nc = _get_program()
    in_maps = make_in_maps(src, tgt, indices, weights,
                           Wq, bq, Wk, bk, Wv, bv, Wo, bo)
    res = run_bass_kernel_spmd(nc, in_maps, core_ids=list(range(8)))
    out = np.empty((B, HW, C), np.float32)
    for core in range(8):
        b, s = divmod(core, 4)
        out[b, s * QL:(s + 1) * QL] = res.results[core]["out"]
    return out

